# revision 4
# baseline (speedup 1.0000x reference)
"""Trainium2 Bass kernel for nn_EquivariantProductBasisBlock (MACE symmetric
contraction, correlation 3, irreps 0e+1o -> 0e+1o, + e3nn linear).

Strategy (data-parallel over nodes, 8 cores):
  Per core: 64 nodes x 64 channels = 4096 (b,c) pairs, each with a 9-dim
  feature vector x.  The full contraction reduces to, per pair:
      T[(D,q)] = sum_f  F[f] * Ucat[f, (D,q)]          (matmul, f = 219)
      f[D]     = sum_q  Wexp[(D,q)] * T[(D,q)]          (species weights)
      out      = blockdiag(Wlin) applied over channels  (matmul)
  where F = [x (9) | sym pairs x_j x_k (45) | sym triples x_i x_j x_k (165)]
  and Ucat folds the (symmetric) U3/U2/U1 CG tensors with permutation
  multiplicities.  Species gather + all weight packing happens host-side.

Device pipeline per core:
  DMA x -> DVE monomials (natural layout [bc, f]) -> PE transpose -> ACT evac
  -> PE matmul vs Ucat -> DVE * Wexp -> PE segmented-sum -> PE blockdiag Wlin
  -> DMA out.
"""

import sys

for _p in ("/opt/trn_rl_repo",):
    if _p not in sys.path:
        sys.path.insert(0, _p)

import numpy as np
import ml_dtypes

N_CORES = 8
N_NODES = 512
B = N_NODES // N_CORES  # nodes per core
C = 64                  # channels
NF = 9                  # features per channel (irreps 0e+1o+2e)
BC = B * C              # 4096 pairs per core
G = BC // 128           # 32 partition tiles
K3, K2, K1 = 16, 4, 1
NQ = K3 + K2 + K1       # 21
ND = 4                  # output dims: idx0 d=1, idx1 d=3
MUL = 64

# Symmetric bases ------------------------------------------------------------
PAIRS = [(j, k) for j in range(NF) for k in range(j, NF)]  # 45, j<=k
TRI2 = {jk: t for t, jk in enumerate(PAIRS)}
NP2 = len(PAIRS)  # 45
# triples (i<=j<=k): stored as per-i segments; segment i = pairs with j>=i,
# which is the contiguous tail of PAIRS starting at TRI2[(i,i)].
SEG_OFF = []  # offset of segment i within the 165 triple block
SEG_LEN = []  # length of segment i
_off = 0
for i in range(NF):
    SEG_OFF.append(_off)
    SEG_LEN.append(NP2 - TRI2[(i, i)])
    _off += SEG_LEN[-1]
NP3 = _off  # 165
NFEAT_TOT = NF + NP2 + NP3  # 219
CH0 = 128
CH1 = NFEAT_TOT - CH0  # 91

F_COL_X = 0
F_COL_P2 = NF          # 9
F_COL_P3 = NF + NP2    # 54

BF16 = ml_dtypes.bfloat16

_CACHE = {}


def _mult3(i, j, k):
    # number of distinct permutations of the multiset {i,j,k}
    if i == j == k:
        return 1.0
    if i == j or j == k or i == k:
        return 3.0
    return 6.0


def _host_pack(node_feats, node_specie,
               U3_0, U2_0, U1_0, w3_0, w2_0, w1_0,
               U3_1, U2_1, U1_1, w3_1, w2_1, w1_1,
               Wlin0, Wlin1):
    """Pack all constant operands host-side. Returns per-core input maps."""
    node_feats = np.asarray(node_feats, np.float32)
    spec = np.asarray(node_specie).astype(np.int64)

    # --- Ucat [219, 84] ---
    ucat = np.zeros((NFEAT_TOT, ND * NQ), np.float32)
    Us = [(np.asarray(U3_0, np.float32), np.asarray(U2_0, np.float32),
           np.asarray(U1_0, np.float32)),
          (np.asarray(U3_1, np.float32), np.asarray(U2_1, np.float32),
           np.asarray(U1_1, np.float32))]
    for D in range(ND):
        idx = 0 if D == 0 else 1
        d = 0 if D == 0 else D - 1
        U3, U2, U1 = Us[idx]
        col = D * NQ
        ucat[F_COL_X:F_COL_X + NF, col + K3 + K2] = U1[d, :, 0]
        for t, (j, k) in enumerate(PAIRS):
            m2 = 1.0 if j == k else 2.0
            ucat[F_COL_P2 + t, col + K3:col + K3 + K2] = m2 * U2[d, j, k, :]
        for i in range(NF):
            for s, (j, k) in enumerate(PAIRS[TRI2[(i, i)]:]):
                r = F_COL_P3 + SEG_OFF[i] + s
                ucat[r, col:col + K3] = _mult3(i, j, k) * U3[d, i, j, k, :]

    # --- per-node species weights, transposed layout [84, b, c] ---
    wcat = np.concatenate([
        np.asarray(w3_0, np.float32), np.asarray(w2_0, np.float32),
        np.asarray(w1_0, np.float32), np.asarray(w3_1, np.float32),
        np.asarray(w2_1, np.float32), np.asarray(w1_1, np.float32),
    ], axis=1)                      # [NSPEC, 42, C]
    wnode = wcat[spec]              # [512, 42, C]

    # --- segment-sum selector [84, 4] ---
    sseg = np.zeros((ND * NQ, ND), np.float32)
    for D in range(ND):
        sseg[D * NQ:(D + 1) * NQ, D] = 1.0

    # --- block-diag Wlin [2, 128, 128] (path norm 1/sqrt(C) folded in) ---
    inv_sqrt_c = 1.0 / np.sqrt(np.float32(C))
    bw = np.zeros((2, 128, 128), np.float32)
    for b2 in range(2):
        bw[0, b2 * 64:(b2 + 1) * 64, b2 * 64:(b2 + 1) * 64] = \
            np.asarray(Wlin0, np.float32) * inv_sqrt_c
        bw[1, b2 * 64:(b2 + 1) * 64, b2 * 64:(b2 + 1) * 64] = \
            np.asarray(Wlin1, np.float32) * inv_sqrt_c

    ident = np.eye(128, dtype=np.float32)

    in_maps = []
    for core in range(N_CORES):
        b0 = core * B
        wex42 = wnode[b0:b0 + B].transpose(1, 0, 2)          # [42, B, C]
        wex84 = np.concatenate(
            [wex42[0:NQ]] + [wex42[NQ:2 * NQ]] * 3, axis=0)  # [84, B, C]
        in_maps.append({
            "x": np.ascontiguousarray(node_feats[b0:b0 + B]),
            "wexp": np.ascontiguousarray(
                wex84.reshape(ND * NQ, BC).astype(BF16)),
            "ucat": ucat.astype(BF16),
            "sseg": sseg.astype(BF16),
            "bw": bw.astype(BF16),
            "ident": ident.astype(BF16),
        })
    return in_maps


def _build_nc():
    import concourse.bass as bass
    import concourse.tile as tile
    from concourse import mybir, bacc

    F32 = mybir.dt.float32
    BF = mybir.dt.bfloat16

    nc = bacc.Bacc("TRN2", target_bir_lowering=False, debug=False,
                   num_devices=N_CORES)

    x_d = nc.dram_tensor("x", [B, C, NF], F32, kind="ExternalInput").ap()
    wexp_d = nc.dram_tensor("wexp", [ND * NQ, BC], BF,
                            kind="ExternalInput").ap()
    ucat_d = nc.dram_tensor("ucat", [NFEAT_TOT, ND * NQ], BF,
                            kind="ExternalInput").ap()
    sseg_d = nc.dram_tensor("sseg", [ND * NQ, ND], BF,
                            kind="ExternalInput").ap()
    bw_d = nc.dram_tensor("bw", [2, 128, 128], BF, kind="ExternalInput").ap()
    ident_d = nc.dram_tensor("ident", [128, 128], BF,
                             kind="ExternalInput").ap()
    out_d = nc.dram_tensor("out", [B, ND * MUL], F32,
                           kind="ExternalOutput").ap()

    NSPLIT = 2           # monomial-formation sub-batches (pipelining)
    GH = G // NSPLIT     # 16 g-tiles per sub-batch
    EB = 8               # g-tiles per transpose/evac batch (one PSUM bank)

    with tile.TileContext(nc) as tc:
        with (
            tc.tile_pool(name="const", bufs=1) as constp,
            tc.tile_pool(name="xin", bufs=1) as xinp,
            tc.tile_pool(name="fnat", bufs=1) as fnatp,
            tc.tile_pool(name="ft", bufs=1) as ftp,
            tc.tile_pool(name="gbuf", bufs=1) as gbufp,
            tc.tile_pool(name="fsb", bufs=1) as fsbp,
            tc.tile_pool(name="tp0", bufs=2, space="PSUM") as tp0p,
            tc.tile_pool(name="tp1", bufs=2, space="PSUM") as tp1p,
            tc.tile_pool(name="tps", bufs=2, space="PSUM") as tpsp,
            tc.tile_pool(name="fps", bufs=1, space="PSUM") as fpsp,
            tc.tile_pool(name="ops", bufs=1, space="PSUM") as opsp,
        ):
            # ---- constants in ----
            u0_sb = constp.tile([CH0, ND * NQ], BF)
            nc.sync.dma_start(u0_sb[:], ucat_d[0:CH0])
            u1_sb = constp.tile([CH1, ND * NQ], BF)
            nc.sync.dma_start(u1_sb[:], ucat_d[CH0:NFEAT_TOT])
            sseg_sb = constp.tile([ND * NQ, ND], BF)
            nc.sync.dma_start(sseg_sb[:], sseg_d)
            bw0_sb = constp.tile([128, 128], BF)
            nc.sync.dma_start(bw0_sb[:], bw_d[0])
            bw1_sb = constp.tile([128, 128], BF)
            nc.sync.dma_start(bw1_sb[:], bw_d[1])
            id_sb = constp.tile([128, 128], BF)
            nc.sync.dma_start(id_sb[:], ident_d)
            wexp_sb = constp.tile([ND * NQ, BC], BF)
            nc.sync.dma_start(wexp_sb[:], wexp_d)

            # ---- node features in: [128=(b2,c), g, i] ----
            x_all = xinp.tile([128, G, NF], F32)
            nc.sync.dma_start(
                x_all[:],
                x_d.rearrange("(g b2) c i -> (b2 c) g i", g=G, b2=2))

            f_nat = fnatp.tile([128, G, NFEAT_TOT], BF)
            y2 = xinp.tile([128, G, NP2], F32)

            # ---- monomial formation (DVE), in NSPLIT sub-batches over g ----
            for h in range(NSPLIT):
                gs = slice(h * GH, (h + 1) * GH)
                xs = x_all[:, gs]
                # copy x into F (bf16 cast)
                nc.vector.tensor_copy(f_nat[:, gs, F_COL_X:F_COL_X + NF], xs)
                # pairs: y2[t(j,k)] = x_j * x_k, j<=k  (fp32 scratch)
                for j in range(NF):
                    n = NF - j
                    t0 = TRI2[(j, j)]
                    nc.vector.tensor_mul(
                        y2[:, gs, t0:t0 + n],
                        xs[:, :, j:j + 1].broadcast_to([128, GH, n]),
                        xs[:, :, j:NF])
                # cast pairs into F
                nc.vector.tensor_copy(f_nat[:, gs, F_COL_P2:F_COL_P2 + NP2],
                                      y2[:, gs])
                # triples: seg i = x_i * y2[t >= t(i,i)]
                for i in range(NF):
                    n = SEG_LEN[i]
                    t0 = TRI2[(i, i)]
                    o = F_COL_P3 + SEG_OFF[i]
                    nc.vector.tensor_mul(
                        f_nat[:, gs, o:o + n],
                        xs[:, :, i:i + 1].broadcast_to([128, GH, n]),
                        y2[:, gs, t0:t0 + NP2 - t0])

            # ---- transpose F to [f, bc] (PE) + evacuate (ACT) ----
            ft0 = ftp.tile([CH0, BC], BF)
            ft1 = ftp.tile([CH1, BC], BF)
            for bi in range(G // EB):
                p0 = tp0p.tile([CH0, EB, 128], BF, tag="tp0")
                p1 = tp1p.tile([CH1, EB, 128], BF, tag="tp1")
                for e in range(EB):
                    g = bi * EB + e
                    nc.tensor.transpose(p0[:, e], f_nat[:, g, 0:CH0], id_sb[:])
                    nc.tensor.transpose(p1[:, e], f_nat[:, g, CH0:NFEAT_TOT],
                                        id_sb[:])
                cols = slice(bi * EB * 128, (bi + 1) * EB * 128)
                nc.scalar.copy(ft0[:, cols], p0[:])
                nc.scalar.copy(ft1[:, cols], p1[:])

            # ---- main contraction + species weights ----
            gb = gbufp.tile([ND * NQ, BC], BF)
            NGRP = 8
            W = BC // NGRP  # 512
            for n in range(NGRP):
                cols = slice(n * W, (n + 1) * W)
                t_ps = tpsp.tile([ND * NQ, W], F32, tag="tps")
                nc.tensor.matmul(t_ps[:], u0_sb[:], ft0[:, cols],
                                 start=True, stop=False)
                nc.tensor.matmul(t_ps[:], u1_sb[:], ft1[:, cols],
                                 start=False, stop=True)
                nc.vector.tensor_mul(gb[:, cols], wexp_sb[:, cols], t_ps[:])

            # ---- segmented sum over q (PE): f[bc, D] ----
            f_ps = fpsp.tile([128, G, ND], F32)
            for g in range(G):
                nc.tensor.matmul(f_ps[:, g], gb[:, g * 128:(g + 1) * 128],
                                 sseg_sb[:], start=True, stop=True)
            f_sb = fsbp.tile([128, G, ND], BF)
            nc.vector.tensor_copy(f_sb[:], f_ps[:])

            # ---- final linear (block-diag Wlin over channels) ----
            o_ps = opsp.tile([128, 128], F32)
            nc.tensor.matmul(o_ps[:, 0:G], bw0_sb[:], f_sb[:, :, 0],
                             start=True, stop=True)
            nc.tensor.matmul(
                o_ps[:, G:G + G * 3].rearrange("p (g i) -> p g i", g=G),
                bw1_sb[:], f_sb[:, :, 1:4], start=True, stop=True)

            # ---- outputs ----
            o_sb = fsbp.tile([128, 128], F32)
            nc.vector.tensor_copy(o_sb[:], o_ps[:])
            # out0: src [p=(b2,M), g] -> out[2g+b2, M]  (one DMA per b2)
            out0 = out_d[:, 0:MUL].rearrange("(g b2) m -> b2 m g", g=G, b2=2)
            out1 = out_d[:, MUL:].rearrange("(g b2) (m i) -> b2 m g i",
                                            g=G, b2=2, i=3)
            for b2 in range(2):
                rows = slice(b2 * 64, (b2 + 1) * 64)
                nc.sync.dma_start(out0[b2], o_sb[rows, 0:G])
                # out1: src [p=M, (g,i)] -> out[2g+b2, 64 + 3M + i]
                nc.sync.dma_start(
                    out1[b2],
                    o_sb[rows, G:G + G * 3].rearrange("m (g i) -> m g i",
                                                      g=G))

    nc.compile()
    return nc


def _get_nc():
    if "nc" not in _CACHE:
        _CACHE["nc"] = _build_nc()
    return _CACHE["nc"]


def kernel(node_feats, node_specie,
           U3_0, U2_0, U1_0, w3_0, w2_0, w1_0,
           U3_1, U2_1, U1_1, w3_1, w2_1, w1_1,
           Wlin0, Wlin1):
    from concourse.bass_utils import run_bass_kernel_spmd

    in_maps = _host_pack(node_feats, node_specie,
                         U3_0, U2_0, U1_0, w3_0, w2_0, w1_0,
                         U3_1, U2_1, U1_1, w3_1, w2_1, w1_1,
                         Wlin0, Wlin1)
    nc = _get_nc()
    res = run_bass_kernel_spmd(nc, in_maps, core_ids=list(range(N_CORES)))
    out = np.concatenate([res.results[k]["out"] for k in range(N_CORES)],
                         axis=0)
    return out.astype(np.float32)
